# revision 58
# baseline (speedup 1.0000x reference)
"""Distributed attention kernel for Trainium2 (8 NeuronCores), Bass/Tile.

Reference computation (B=2, S=2048, D=768, N=12, H=64):
  q/k/v = per-head projections of x_q / x_kv; LayerNorm(H) on q, k;
  causal SDPA (scale=1); per-head output projection summed over heads.

Sharding: 8 cores = batch (2) x head-groups (4 groups of 3 heads).
Each core computes a full (S, D) partial output for its (batch, 3 heads);
host sums the 4 partials per batch.

Per-core kernel design (fp16 matmul operands, fp32 accumulation; the
softmax probabilities and their reciprocals are bf16 for dynamic range):
  - Host pre-transposes activations to X^T (contraction dim on SBUF
    partitions) and pre-packs Q|K|V weights; all inputs fp16.
  - QKV projection in natural layout [s, h]: lhsT = X^T tile (stationary),
    rhs = packed W. Q/K/V accumulate in separate PSUM banks (start=True
    clears has_written for a whole bank).
  - LayerNorm per head with batched stats (reduce over free dim) + fused
    (x-mean)*rstd via tensor_scalar; the q-side LN overlaps the KV
    projection pass, and the k-side LN is split into s-tile halves so
    j=0 attention starts before the kv pass fully drains.
  - PE transposes Qn/Kn into Q^T/K^T [64, 512] group tiles (h on
    partitions) so attention depends on exactly the slices it reads.
  - Scores computed transposed, S^T[k,q] = K @ Q^T per 128-row k-tile
    over 1024-wide q-blocks; causal: fully-masked tiles skipped, the
    diagonal subblock gets an additive -30k triangular mask via an extra
    accumulating matmul. No max-subtraction (|scores| <~ 50 is safe in
    fp32; exp is evaluated in fp32 out of PSUM).
  - exp on ScalarE -> P^T bf16; PV matmul with V' = [V | ones] so row 64
    of Z'^T accumulates the softmax denominators for free.
  - Denominator reciprocals: l rows -> K=1 outer-product matmuls onto
    [128, 24] columns -> one DVE reciprocal -> PE transpose back ->
    selector-matmul broadcast across partitions (PSUM) -> one
    tensor_tensor multiply normalizes Z^T (fp16).
  - Output projection: Z^T head-stacked [128+64, s] as lhsT, Wo as rhs,
    heads accumulated in PSUM; partial (S, D) fp32 DMA'd out.

This walrus build honors only one embedded sem-wait per instruction, so
_split_multi_waits() hoists extra waits into standalone EventSemaphore
instructions after Tile scheduling. Timing comes from the Bass
instruction-cost timeline simulator (no NTFF path on this axon rig).
"""
import numpy as np

B, S, D, N, H = 2, 2048, 768, 12, 64
EPS = 1e-5
N_CORES = 8
HPC = 3          # heads per core
KT = D // 128    # 6 k-tiles over the d_model contraction
ST = S // 128    # 16 s-tiles
QB = 1024        # q-block width for attention phase
NQB = S // QB    # 2
KPB = QB // 128  # 8 k-tiles per q-block span
NEG = -30000.0

_cache = {}


def _split_multi_waits(nc, mybir):
    """Walrus in this env only honors one embedded sem-wait per instruction;
    hoist extras into standalone EventSemaphore waits on the same engine.
    Safe: Tile's schedule is consistent with per-engine program order, so
    blocking the engine at the instruction instead of attaching the wait
    cannot deadlock."""
    for bb in nc.main_func.blocks:
        new, changed = [], False
        for ins in bb.instructions:
            si = ins.sync_info
            if si is not None and si.on_wait and len(si.on_wait) > 1:
                waits = list(si.on_wait)
                for k, w in enumerate(waits[:-1]):
                    new.append(mybir.InstEventSemaphore(
                        name=f"{ins.name}-sw{k}", engine=ins.engine,
                        ins=[], outs=[],
                        sync_info=mybir.SyncInfo(on_wait=[w], on_update=[])))
                ins.sync_info = mybir.SyncInfo(
                    on_wait=[waits[-1]], on_update=list(si.on_update or []))
                changed = True
            new.append(ins)
        if changed:
            bb.instructions = new


def _build(apply_gb: bool, dump: bool = False):
    """Build the SPMD Bass program (same program for all 8 cores)."""
    from contextlib import ExitStack

    import concourse.bass as bass
    import concourse.tile as tile
    from concourse import mybir

    f32 = mybir.dt.float32
    bf16 = mybir.dt.bfloat16
    f16 = mybir.dt.float16

    nc = bass.Bass()

    xqT = nc.dram_tensor("xqT", [KT, 128, S], f16, kind="ExternalInput")
    xkvT = nc.dram_tensor("xkvT", [KT, 128, S], f16, kind="ExternalInput")
    wqkv = nc.dram_tensor("wqkv", [KT, 128, 576], f16, kind="ExternalInput")
    wo01 = nc.dram_tensor("wo01", [128, D], f16, kind="ExternalInput")
    wo2 = nc.dram_tensor("wo2", [64, D], f16, kind="ExternalInput")
    ident = nc.dram_tensor("ident", [128, 128], f16, kind="ExternalInput")
    identb = nc.dram_tensor("identb", [128, 128], bf16, kind="ExternalInput")
    tmask = nc.dram_tensor("tmask", [128, 128], f16, kind="ExternalInput")
    seld = nc.dram_tensor("seld", [24, 24, 64], bf16, kind="ExternalInput")
    if apply_gb:
        lngb = nc.dram_tensor("lngb", [4, 64, 1], f32, kind="ExternalInput")
    outp = nc.dram_tensor("outp", [S, D], f32, kind="ExternalOutput")

    Exp = mybir.ActivationFunctionType.Exp
    Ln = mybir.ActivationFunctionType.Ln
    Sqrt = mybir.ActivationFunctionType.Sqrt
    sub = mybir.AluOpType.subtract
    mult = mybir.AluOpType.mult
    addop = mybir.AluOpType.add

    with ExitStack() as ctx:
        tc = ctx.enter_context(tile.TileContext(nc))
        const = ctx.enter_context(tc.tile_pool(name="const", bufs=1))

        # ---- resident SBUF tensors (split so consumers depend on exactly
        # the producer they need, not a whole fused buffer) ----
        xq_t = [const.tile([128, S], f16, name=f"xq{k}", tag=f"xq{k}")
                for k in range(KT)]
        xkv_t = [const.tile([128, S], f16, name=f"xkv{k}", tag=f"xkv{k}")
                 for k in range(KT)]
        w_t = [const.tile([128, 576], f16, name=f"w{k}", tag=f"w{k}")
               for k in range(KT)]
        wo01_sb = const.tile([128, D], f16)
        wo2_sb = const.tile([64, D], f16)
        id_sb = const.tile([128, 128], f16)
        idb_sb = const.tile([128, 128], bf16)
        tm_sb = const.tile([128, 128], f16)
        eps_sb = const.tile([128, 1], f32)
        qt_g = [[const.tile([64, 512], f16, name=f"qt{h}_{g}", tag=f"qt{h}_{g}")
                 for g in range(4)] for h in range(HPC)]
        kt_g = [[const.tile([64, 512], f16, name=f"kt{h}_{g}", tag=f"kt{h}_{g}")
                 for g in range(4)] for h in range(HPC)]
        vv_sb = const.tile([128, ST, HPC, 65], bf16)  # V' tiles, col 64 = 1
        z01_jh = [[const.tile([128, 512], f16, name=f"z01_{j}_{hf}",
                              tag=f"z01_{j}_{hf}") for hf in range(2)]
                  for j in range(NQB)]
        z2_jh = [[const.tile([64, 512], f16, name=f"z2_{j}_{hf}",
                             tag=f"z2_{j}_{hf}") for hf in range(2)]
                 for j in range(NQB)]
        zu01_j = [const.tile([128, QB], f32, name=f"zu01_{j}", tag=f"zu01_{j}") for j in range(NQB)]
        zu2_j = [const.tile([64, QB], f32, name=f"zu2_{j}", tag=f"zu2_{j}") for j in range(NQB)]
        q_nat = const.tile([128, ST, 192], f16)
        k_nat = const.tile([128, ST, 192], f16)
        l_hj = [const.tile([1, QB], f32, name=f"l{i}", tag=f"l{i}")
                for i in range(HPC * NQB)]
        sel_sb = const.tile([24, 24, 64], bf16)
        ones1_sb = const.tile([1, 1], f32)

        # one FIFO: xq streams at full queue bandwidth, xkv behind it
        # (kv pass starts ~20us in); weights on the ACT queue in parallel
        for kt in range(KT):
            nc.scalar.dma_start(out=w_t[kt][:], in_=wqkv[kt])
            nc.sync.dma_start(out=xq_t[kt][:], in_=xqT[kt])
        for kt in range(KT):
            nc.sync.dma_start(out=xkv_t[kt][:], in_=xkvT[kt])
        nc.sync.dma_start(out=wo01_sb[:], in_=wo01[:])
        nc.sync.dma_start(out=wo2_sb[:], in_=wo2[:])
        nc.sync.dma_start(out=id_sb[:], in_=ident[:])
        nc.sync.dma_start(out=idb_sb[:], in_=identb[:])
        nc.sync.dma_start(out=tm_sb[:], in_=tmask[:])
        nc.vector.memset(eps_sb[:], EPS)
        nc.vector.memset(vv_sb[:, :, :, 64:65], 1.0)
        nc.vector.memset(ones1_sb[:], 1.0)
        nc.sync.dma_start(out=sel_sb[:], in_=seld[:])
        if apply_gb:
            gb_sb = const.tile([64, 4], f32)
            for i in range(4):
                nc.sync.dma_start(out=gb_sb[:, i : i + 1], in_=lngb[i])

        def ln_head(stat, qn_pool, tp_ps, side, nat, dst_h, h, jbs=(0, 1, 2, 3)):
            """LayerNorm stats + fused apply + PE transpose for one head,
            restricted to the given jb groups (4 s-tiles each). Stats are
            per (row, tile), so partial ranges are exact."""
            t0, t1 = 4 * jbs[0], 4 * jbs[-1] + 4
            nt = t1 - t0
            hsl = nat[:, t0:t1, 64 * h : 64 * (h + 1)]
            sq = stat.tile([128, nt, 64], f32, tag=f"sq{len(jbs)}")
            nc.scalar.square(sq[:], hsl)
            ssum = stat.tile([128, nt], f32, tag=f"ssum{len(jbs)}")
            ssq = stat.tile([128, nt], f32, tag=f"ssq{len(jbs)}")
            nc.vector.tensor_reduce(
                ssum[:], hsl, axis=mybir.AxisListType.X, op=addop)
            nc.vector.tensor_reduce(
                ssq[:], sq[:], axis=mybir.AxisListType.X, op=addop)
            mean = stat.tile([128, nt], f32, tag=f"mean{len(jbs)}")
            rstd = stat.tile([128, nt], f32, tag=f"rstd{len(jbs)}")
            var = stat.tile([128, nt], f32, tag=f"var{len(jbs)}")
            nc.vector.tensor_scalar_mul(mean[:], ssum[:], 1.0 / H)
            nc.vector.tensor_mul(var[:], mean[:], mean[:])
            nc.vector.scalar_tensor_tensor(
                out=var[:], in0=ssq[:], scalar=1.0 / H, in1=var[:],
                op0=mult, op1=sub)
            nc.scalar.activation(var[:], var[:], Sqrt, bias=eps_sb[:])
            nc.vector.reciprocal(rstd[:], var[:])
            for jb in jbs:
                tp = tp_ps.tile([64, 512], f16)
                for tt in range(4):
                    t = 4 * jb + tt
                    qn = qn_pool.tile([128, 64], f16)
                    nc.vector.tensor_scalar(
                        out=qn[:], in0=nat[:, t, 64 * h : 64 * h + 64],
                        scalar1=mean[:, t - t0 : t - t0 + 1],
                        scalar2=rstd[:, t - t0 : t - t0 + 1],
                        op0=sub, op1=mult)
                    nc.tensor.transpose(
                        tp[:, 128 * tt : 128 * (tt + 1)], qn[:], id_sb[:])
                dsl = dst_h[h][jb][:]
                if apply_gb:
                    nc.vector.tensor_scalar(
                        out=dsl, in0=tp[:],
                        scalar1=gb_sb[:, 2 * side : 2 * side + 1],
                        scalar2=gb_sb[:, 2 * side + 1 : 2 * side + 2],
                        op0=mult, op1=addop)
                else:
                    nc.any.tensor_copy(dsl, tp[:])

        with tc.tile_pool(name="stat", bufs=2) as stat, \
             tc.tile_pool(name="qn", bufs=6) as qn_pool, \
             tc.tile_pool(name="pt", bufs=4) as pt_pool, \
             tc.tile_pool(name="rr", bufs=2) as rr_pool:
            # ===== Phase 1: projections; q-side LN overlaps the KV pass ====
            with tc.tile_pool(name="proj_ps", bufs=2, space="PSUM") as proj_ps, \
                 tc.tile_pool(name="tp_ps", bufs=2, space="PSUM") as tp_ps:
                for t in range(ST):
                    psq = proj_ps.tile([128, 192], f32, tag="psq")
                    sl = slice(128 * t, 128 * (t + 1))
                    for kt in range(KT):
                        nc.tensor.matmul(
                            psq[:], xq_t[kt][:, sl], w_t[kt][:, 0:192],
                            start=(kt == 0), stop=(kt == KT - 1))
                    nc.any.tensor_copy(q_nat[:, t, :], psq[:])
                for h in range(HPC):
                    ln_head(stat, qn_pool, tp_ps, 0, q_nat, qt_g, h)
                for t in range(ST):
                    pskv = proj_ps.tile([128, 384], f32, tag="pskv")
                    sl = slice(128 * t, 128 * (t + 1))
                    for kt in range(KT):
                        nc.tensor.matmul(
                            pskv[:], xkv_t[kt][:, sl], w_t[kt][:, 192:576],
                            start=(kt == 0), stop=(kt == KT - 1))
                    nc.any.tensor_copy(k_nat[:, t, :], pskv[:, 0:192])
                    nc.any.tensor_copy(
                        vv_sb[:, t, :, 0:64],
                        pskv[:, 192:384].rearrange("p (h d) -> p h d", h=HPC))
                # k-side LN for s-tiles 0..7 (all j=0 attention needs) --
                # overlaps the tail of the kv pass; half1 follows for j=1
                for h in range(HPC):
                    ln_head(stat, qn_pool, tp_ps, 1, k_nat, kt_g, h,
                            jbs=(0, 1))
                for h in range(HPC):
                    ln_head(stat, qn_pool, tp_ps, 1, k_nat, kt_g, h,
                            jbs=(2, 3))

            # ===== Phase 2a: attention (k-side LN already emitted) =====
            with tc.tile_pool(name="st_ps", bufs=3, space="PSUM") as st_ps, \
                 tc.tile_pool(name="zp_ps", bufs=1, space="PSUM") as zp_ps:

                def attn_block(j, h):
                    q0 = QB * j
                    zp = zp_ps.tile([65, QB], f32)
                    n_kt = (j + 1) * KPB

                    def emit_scores(i):
                        off = max(0, 128 * i - q0)   # first valid col
                        diag = 128 * i >= q0
                        stp = st_ps.tile([128, QB], f32)
                        for b0 in range(0, QB, 512):
                            c0, c1 = max(off, b0), b0 + 512
                            if c0 >= c1:
                                continue
                            mask_here = diag and b0 <= off < b0 + 512
                            qg, qo = (q0 + c0) // 512, (q0 + c0) % 512
                            nc.tensor.matmul(
                                stp[:, c0:c1],
                                kt_g[h][i // 4][:, 128 * (i % 4) : 128 * (i % 4 + 1)],
                                qt_g[h][qg][:, qo : qo + (c1 - c0)],
                                start=True, stop=not mask_here)
                            if mask_here:
                                nc.tensor.matmul(
                                    stp[:, off : off + 128], tm_sb[:], id_sb[:],
                                    start=False, stop=True,
                                    skip_group_check=True)
                        return stp, off

                    def emit_exp_pv(i, stp, off):
                        pt = pt_pool.tile([128, QB], bf16)
                        nc.scalar.activation(pt[:, off:], stp[:, off:], Exp)
                        for b0 in range(0, QB, 512):
                            c0, c1 = max(off, b0), b0 + 512
                            if c0 >= c1:
                                continue
                            last_i = min(n_kt, j * KPB + b0 // 128 + 4) - 1
                            nc.tensor.matmul(
                                zp[:, c0:c1], vv_sb[:, i, h, :], pt[:, c0:c1],
                                start=(i == 0), stop=(i == last_i),
                                skip_group_check=True)

                    # software pipeline: scores(i) issued before exp/PV(i-1)
                    prev = None
                    for i in range(n_kt):
                        cur = emit_scores(i)
                        if prev is not None:
                            emit_exp_pv(prev[0], prev[1], prev[2])
                        prev = (i, cur[0], cur[1])
                    emit_exp_pv(prev[0], prev[1], prev[2])
                    # denominators: evict row 64 (r computed in the tail)
                    nc.vector.tensor_copy(l_hj[h * NQB + j][:], zp[64:65, :])
                    # unnormalized Z~^T -> SBUF fp32
                    zdst = (zu01_j[j][64 * h : 64 * (h + 1), :]
                            if h < 2 else zu2_j[j][:, :])
                    nc.any.tensor_copy(zdst, zp[0:64, :])

                for h in range(HPC):
                    attn_block(0, h)
                for h in range(HPC):
                    attn_block(1, h)

        # ====== Phase 2b + 3: normalize Z^T and output projection ======
        # one reciprocal chain per j: l rows -> PE outer products -> [128, 24]
        # columns -> DVE reciprocal -> PE transpose -> selector-matmul bcast
        with tc.tile_pool(name="lc_ps", bufs=1, space="PSUM") as lc_ps, \
             tc.tile_pool(name="r8_ps", bufs=1, space="PSUM") as r8_ps, \
             tc.tile_pool(name="rbc_ps", bufs=2, space="PSUM") as rbc_ps, \
             tc.tile_pool(name="op_ps", bufs=2, space="PSUM") as op_ps, \
             tc.tile_pool(name="rr2", bufs=2) as rr2_pool, \
             tc.tile_pool(name="ot", bufs=4) as ot_pool:
            for j in range(NQB):
                lcol = lc_ps.tile([128, 24], f32)
                for h in range(HPC):
                    ll = l_hj[h * NQB + j]
                    for b in range(8):
                        nc.tensor.matmul(
                            lcol[:, 8 * h + b : 8 * h + b + 1],
                            ll[0:1, 128 * b : 128 * (b + 1)], ones1_sb[:],
                            start=True, stop=True, skip_group_check=True)
                rsbf = rr2_pool.tile([128, 24], f32, tag="rsbf")
                rsb = rr2_pool.tile([128, 24], bf16, tag="rsb")
                nc.vector.reciprocal(rsbf[:], lcol[:])
                nc.vector.tensor_copy(rsb[:], rsbf[:])
                r8p = r8_ps.tile([24, 128], bf16)
                nc.tensor.transpose(r8p[:], rsb[:], idb_sb[:])
                r8 = rr2_pool.tile([24, 128], bf16, tag="r8")
                nc.vector.tensor_copy(r8[:], r8p[:])
                for h in range(HPC):
                    p0 = 64 * (h % 2)
                    zsrc = (zu01_j[j][64 * h : 64 * (h + 1), :]
                            if h < 2 else zu2_j[j][:, :])
                    for half in range(2):  # 512-wide chunks, 1 psum bank each
                        cs = slice(512 * half, 512 * (half + 1))
                        zdst = (z01_jh[j][half][64 * h : 64 * (h + 1), :]
                                if h < 2 else z2_jh[j][half][:, :])
                        rbc = rbc_ps.tile([128, 512], f32)
                        for bb in range(4):
                            b = 8 * h + 4 * half + bb
                            nc.tensor.matmul(
                                rbc[p0 : p0 + 64, 128 * bb : 128 * (bb + 1)],
                                sel_sb[:, b, :], r8[:],
                                start=True, stop=True, skip_group_check=True)
                        nc.vector.tensor_tensor(
                            out=zdst, in0=zsrc[:, cs],
                            in1=rbc[p0 : p0 + 64, :], op=mult)
                # output projection for this j's s-tiles
                for mm in range(QB // 128):
                    m = j * (QB // 128) + mm
                    sl = slice(128 * m, 128 * (m + 1))
                    op = op_ps.tile([128, D], f32)
                    zh, zo = mm // 4, 128 * (mm % 4)
                    for c0 in range(0, D, 512):
                        c1 = min(c0 + 512, D)
                        nc.tensor.matmul(
                            op[:, c0:c1], z01_jh[j][zh][:, zo : zo + 128],
                            wo01_sb[:, c0:c1], start=True, stop=False)
                        nc.tensor.matmul(
                            op[:, c0:c1], z2_jh[j][zh][:, zo : zo + 128],
                            wo2_sb[:, c0:c1], start=False, stop=True)
                    ot = ot_pool.tile([128, D], f32)
                    nc.scalar.copy(ot[:], op[:])
                    nc.sync.dma_start(out=outp[sl, :], in_=ot[:])

        if dump:
            for name, t in [
                ("d_qnat", q_nat), ("d_knat", k_nat), ("d_vv", vv_sb),
            ]:
                dt = nc.dram_tensor(name, list(t.shape), t.dtype,
                                    kind="ExternalOutput")
                nc.sync.dma_start(out=dt[:], in_=t[:])
            for pre, tl in [("d_zu01", zu01_j),
                            ("d_zu2", zu2_j)]:
                for i, t in enumerate(tl):
                    dt = nc.dram_tensor(f"{pre}_{i}", list(t.shape), t.dtype,
                                        kind="ExternalOutput")
                    nc.sync.dma_start(out=dt[:], in_=t[:])

    _split_multi_waits(nc, mybir)
    return nc


def _get_nc(apply_gb: bool):
    key = ("nc", apply_gb)
    if key not in _cache:
        _cache[key] = _build(apply_gb)
    return _cache[key]


def _in_maps(x_q, x_kv, W_Q, W_K, W_V, W_O, ln1_g, ln1_b, ln2_g, ln2_b, apply_gb):
    import ml_dtypes

    bf16 = ml_dtypes.bfloat16
    f16 = np.float16
    ident = np.eye(128, dtype=np.float32).astype(f16)
    identb = np.eye(128, dtype=np.float32).astype(bf16)
    tm = np.where(np.triu(np.ones((128, 128), dtype=bool), 1), NEG, 0.0).astype(f16)
    sel24 = np.ascontiguousarray(
        np.repeat(np.eye(24, dtype=np.float32)[:, :, None], 64, axis=2)).astype(bf16)
    maps = []
    for c in range(N_CORES):
        b, g = divmod(c, 4)
        hs = slice(HPC * g, HPC * (g + 1))
        wq = W_Q[hs].transpose(1, 0, 2).reshape(D, HPC * H)
        wk = W_K[hs].transpose(1, 0, 2).reshape(D, HPC * H)
        wv = W_V[hs].transpose(1, 0, 2).reshape(D, HPC * H)
        m = {
            "xqT": np.ascontiguousarray(x_q[b].T).astype(f16).reshape(KT, 128, S),
            "xkvT": np.ascontiguousarray(x_kv[b].T).astype(f16).reshape(KT, 128, S),
            "wqkv": np.ascontiguousarray(np.concatenate([wq, wk, wv], axis=1))
                      .astype(f16).reshape(KT, 128, 576),
            "wo01": np.ascontiguousarray(W_O[hs][0:2].reshape(128, D)).astype(f16),
            "wo2": np.ascontiguousarray(W_O[hs][2]).astype(f16),
            "ident": ident,
            "identb": identb,
            "tmask": tm,
            "seld": sel24,
        }
        if apply_gb:
            m["lngb"] = np.stack([ln1_g, ln1_b, ln2_g, ln2_b]) \
                          .astype(np.float32).reshape(4, 64, 1)
        maps.append(m)
    return maps


def _run(inputs, trace=False):
    from concourse.bass_utils import run_bass_kernel_spmd

    x_q = np.asarray(inputs["x_q"], np.float32)
    x_kv = np.asarray(inputs["x_kv"], np.float32)
    W_Q = np.asarray(inputs["W_Q"], np.float32)
    W_K = np.asarray(inputs["W_K"], np.float32)
    W_V = np.asarray(inputs["W_V"], np.float32)
    W_O = np.asarray(inputs["W_O"], np.float32)
    g1 = np.asarray(inputs["ln1_g"], np.float32)
    b1 = np.asarray(inputs["ln1_b"], np.float32)
    g2 = np.asarray(inputs["ln2_g"], np.float32)
    b2 = np.asarray(inputs["ln2_b"], np.float32)

    apply_gb = not (
        np.all(g1 == 1) and np.all(g2 == 1) and np.all(b1 == 0) and np.all(b2 == 0)
    )
    nc = _get_nc(apply_gb)
    maps = _in_maps(x_q, x_kv, W_Q, W_K, W_V, W_O, g1, b1, g2, b2, apply_gb)
    res = run_bass_kernel_spmd(nc, maps, list(range(N_CORES)), trace=trace)
    out = np.zeros((B, S, D), np.float32)
    for c in range(N_CORES):
        out[c // 4] += np.asarray(res.results[c]["outp"], np.float32)
    return out, res


def kernel(x_q, x_kv, mask, W_Q, W_K, W_V, W_O, ln1_g, ln1_b, ln2_g, ln2_b):
    out, _ = _run(dict(
        x_q=x_q, x_kv=x_kv, W_Q=W_Q, W_K=W_K, W_V=W_V, W_O=W_O,
        ln1_g=ln1_g, ln1_b=ln1_b, ln2_g=ln2_g, ln2_b=ln2_b))
    return out


def kernel_profiled(x_q, x_kv, mask, W_Q, W_K, W_V, W_O,
                    ln1_g, ln1_b, ln2_g, ln2_b):
    """Runs on HW for correctness; returns (out, exec_time_ns).

    exec_time_ns is the per-core device execution time from the official
    Bass instruction-cost timeline simulator (concourse.timeline_sim) --
    this axon-tunneled rig exposes no NTFF/neuron-profile path, and all 8
    cores run the same program in parallel, so kernel time = per-core time.
    """
    out, res = _run(dict(
        x_q=x_q, x_kv=x_kv, W_Q=W_Q, W_K=W_K, W_V=W_V, W_O=W_O,
        ln1_g=ln1_g, ln1_b=ln1_b, ln2_g=ln2_g, ln2_b=ln2_b))
    from concourse.timeline_sim import TimelineSim
    g1 = np.asarray(ln1_g); b1 = np.asarray(ln1_b)
    g2 = np.asarray(ln2_g); b2 = np.asarray(ln2_b)
    apply_gb = not (np.all(g1 == 1) and np.all(g2 == 1)
                    and np.all(b1 == 0) and np.all(b2 == 0))
    exec_ns = TimelineSim(_get_nc(apply_gb), trace=False).simulate()
    return out, int(exec_ns)


# revision 61
# speedup vs baseline: 1.0014x; 1.0014x over previous
"""Distributed attention kernel for Trainium2 (8 NeuronCores), Bass/Tile.

Reference computation (B=2, S=2048, D=768, N=12, H=64):
  q/k/v = per-head projections of x_q / x_kv; LayerNorm(H) on q, k;
  causal SDPA (scale=1); per-head output projection summed over heads.

Sharding: 8 cores = batch (2) x head-groups (4 groups of 3 heads).
Each core computes a full (S, D) partial output for its (batch, 3 heads);
host sums the 4 partials per batch.

Per-core kernel design (fp16 matmul operands, fp32 accumulation; the
softmax probabilities and their reciprocals are bf16 for dynamic range):
  - Host pre-transposes activations to X^T (contraction dim on SBUF
    partitions) and pre-packs Q|K|V weights; all inputs fp16.
  - QKV projection in natural layout [s, h]: lhsT = X^T tile (stationary),
    rhs = packed W. Q/K/V accumulate in separate PSUM banks (start=True
    clears has_written for a whole bank).
  - LayerNorm per head with batched stats (reduce over free dim) + fused
    (x-mean)*rstd via tensor_scalar; the q-side LN overlaps the KV
    projection pass, and the k-side LN is split into s-tile halves so
    j=0 attention starts before the kv pass fully drains.
  - PE transposes Qn/Kn into Q^T/K^T [64, 512] group tiles (h on
    partitions) so attention depends on exactly the slices it reads.
  - Scores computed transposed, S^T[k,q] = K @ Q^T per 128-row k-tile
    over 1024-wide q-blocks; causal: fully-masked tiles skipped, the
    diagonal subblock gets an additive -30k triangular mask via an extra
    accumulating matmul. No max-subtraction (|scores| <~ 50 is safe in
    fp32; exp is evaluated in fp32 out of PSUM).
  - exp on ScalarE -> P^T bf16; PV matmul with V' = [V | ones] so row 64
    of Z'^T accumulates the softmax denominators for free.
  - Denominator reciprocals: l rows -> K=1 outer-product matmuls onto
    [128, 24] columns -> one DVE reciprocal -> PE transpose back ->
    selector-matmul broadcast across partitions (PSUM) -> one
    tensor_tensor multiply normalizes Z^T (fp16).
  - Output projection: Z^T head-stacked [128+64, s] as lhsT, Wo as rhs,
    heads accumulated in PSUM; partial (S, D) fp32 DMA'd out.

This walrus build honors only one embedded sem-wait per instruction, so
_split_multi_waits() hoists extra waits into standalone EventSemaphore
instructions after Tile scheduling. Timing comes from the Bass
instruction-cost timeline simulator (no NTFF path on this axon rig).
"""
import numpy as np

B, S, D, N, H = 2, 2048, 768, 12, 64
EPS = 1e-5
N_CORES = 8
HPC = 3          # heads per core
KT = D // 128    # 6 k-tiles over the d_model contraction
ST = S // 128    # 16 s-tiles
QB = 1024        # q-block width for attention phase
NQB = S // QB    # 2
KPB = QB // 128  # 8 k-tiles per q-block span
NEG = -30000.0

_cache = {}


def _split_multi_waits(nc, mybir):
    """Walrus in this env only honors one embedded sem-wait per instruction;
    hoist extras into standalone EventSemaphore waits on the same engine.
    Safe: Tile's schedule is consistent with per-engine program order, so
    blocking the engine at the instruction instead of attaching the wait
    cannot deadlock."""
    for bb in nc.main_func.blocks:
        new, changed = [], False
        for ins in bb.instructions:
            si = ins.sync_info
            if si is not None and si.on_wait and len(si.on_wait) > 1:
                waits = list(si.on_wait)
                for k, w in enumerate(waits[:-1]):
                    new.append(mybir.InstEventSemaphore(
                        name=f"{ins.name}-sw{k}", engine=ins.engine,
                        ins=[], outs=[],
                        sync_info=mybir.SyncInfo(on_wait=[w], on_update=[])))
                ins.sync_info = mybir.SyncInfo(
                    on_wait=[waits[-1]], on_update=list(si.on_update or []))
                changed = True
            new.append(ins)
        if changed:
            bb.instructions = new


def _build(apply_gb: bool, dump: bool = False):
    """Build the SPMD Bass program (same program for all 8 cores)."""
    from contextlib import ExitStack

    import concourse.bass as bass
    import concourse.tile as tile
    from concourse import mybir

    f32 = mybir.dt.float32
    bf16 = mybir.dt.bfloat16
    f16 = mybir.dt.float16

    nc = bass.Bass()

    xqT = nc.dram_tensor("xqT", [KT, 128, S], f16, kind="ExternalInput")
    xkvT = nc.dram_tensor("xkvT", [KT, 128, S], f16, kind="ExternalInput")
    wqkv = nc.dram_tensor("wqkv", [KT, 128, 576], f16, kind="ExternalInput")
    wo01 = nc.dram_tensor("wo01", [128, D], f16, kind="ExternalInput")
    wo2 = nc.dram_tensor("wo2", [64, D], f16, kind="ExternalInput")
    ident = nc.dram_tensor("ident", [128, 128], f16, kind="ExternalInput")
    identb = nc.dram_tensor("identb", [128, 128], bf16, kind="ExternalInput")
    tmask = nc.dram_tensor("tmask", [128, 128], f16, kind="ExternalInput")
    seld = nc.dram_tensor("seld", [24, 24, 64], bf16, kind="ExternalInput")
    if apply_gb:
        lngb = nc.dram_tensor("lngb", [4, 64, 1], f32, kind="ExternalInput")
    outp = nc.dram_tensor("outp", [S, D], f32, kind="ExternalOutput")

    Exp = mybir.ActivationFunctionType.Exp
    Ln = mybir.ActivationFunctionType.Ln
    Sqrt = mybir.ActivationFunctionType.Sqrt
    sub = mybir.AluOpType.subtract
    mult = mybir.AluOpType.mult
    addop = mybir.AluOpType.add

    with ExitStack() as ctx:
        tc = ctx.enter_context(tile.TileContext(nc))
        const = ctx.enter_context(tc.tile_pool(name="const", bufs=1))

        # ---- resident SBUF tensors (split so consumers depend on exactly
        # the producer they need, not a whole fused buffer) ----
        xq_t = [const.tile([128, S], f16, name=f"xq{k}", tag=f"xq{k}")
                for k in range(KT)]
        xkv_t = [const.tile([128, S], f16, name=f"xkv{k}", tag=f"xkv{k}")
                 for k in range(KT)]
        w_t = [const.tile([128, 576], f16, name=f"w{k}", tag=f"w{k}")
               for k in range(KT)]
        wo01_sb = const.tile([128, D], f16)
        wo2_sb = const.tile([64, D], f16)
        id_sb = const.tile([128, 128], f16)
        idb_sb = const.tile([128, 128], bf16)
        tm_sb = const.tile([128, 128], f16)
        eps_sb = const.tile([128, 1], f32)
        qt_g = [[const.tile([64, 512], f16, name=f"qt{h}_{g}", tag=f"qt{h}_{g}")
                 for g in range(4)] for h in range(HPC)]
        kt_g = [[const.tile([64, 512], f16, name=f"kt{h}_{g}", tag=f"kt{h}_{g}")
                 for g in range(4)] for h in range(HPC)]
        vv_sb = const.tile([128, ST, HPC, 65], bf16)  # V' tiles, col 64 = 1
        z01_jh = [[const.tile([128, 512], f16, name=f"z01_{j}_{hf}",
                              tag=f"z01_{j}_{hf}") for hf in range(2)]
                  for j in range(NQB)]
        z2_jh = [[const.tile([64, 512], f16, name=f"z2_{j}_{hf}",
                             tag=f"z2_{j}_{hf}") for hf in range(2)]
                 for j in range(NQB)]
        zu01_j = [const.tile([128, QB], f32, name=f"zu01_{j}", tag=f"zu01_{j}") for j in range(NQB)]
        zu2_j = [const.tile([64, QB], f32, name=f"zu2_{j}", tag=f"zu2_{j}") for j in range(NQB)]
        q_nat = const.tile([128, ST, 192], f16)
        k_nat = const.tile([128, ST, 192], f16)
        l_hj = [const.tile([1, QB], f32, name=f"l{i}", tag=f"l{i}")
                for i in range(HPC * NQB)]
        sel_sb = const.tile([24, 24, 64], bf16)
        ones1_sb = const.tile([1, 1], f32)

        # one FIFO: xq streams at full queue bandwidth, xkv behind it
        # (kv pass starts ~20us in); weights on the ACT queue in parallel
        for kt in range(KT):
            nc.scalar.dma_start(out=w_t[kt][:], in_=wqkv[kt])
            nc.sync.dma_start(out=xq_t[kt][:], in_=xqT[kt])
        for kt in range(KT):
            nc.sync.dma_start(out=xkv_t[kt][:], in_=xkvT[kt])
        nc.sync.dma_start(out=wo01_sb[:], in_=wo01[:])
        nc.sync.dma_start(out=wo2_sb[:], in_=wo2[:])
        nc.sync.dma_start(out=id_sb[:], in_=ident[:])
        nc.sync.dma_start(out=idb_sb[:], in_=identb[:])
        nc.sync.dma_start(out=tm_sb[:], in_=tmask[:])
        nc.vector.memset(eps_sb[:], EPS)
        nc.vector.memset(vv_sb[:, :, :, 64:65], 1.0)
        nc.vector.memset(ones1_sb[:], 1.0)
        nc.sync.dma_start(out=sel_sb[:], in_=seld[:])
        if apply_gb:
            gb_sb = const.tile([64, 4], f32)
            for i in range(4):
                nc.sync.dma_start(out=gb_sb[:, i : i + 1], in_=lngb[i])

        def ln_head(stat, qn_pool, tp_ps, side, nat, dst_h, h, jbs=(0, 1, 2, 3)):
            """LayerNorm stats + fused apply + PE transpose for one head,
            restricted to the given jb groups (4 s-tiles each). Stats are
            per (row, tile), so partial ranges are exact."""
            t0, t1 = 4 * jbs[0], 4 * jbs[-1] + 4
            nt = t1 - t0
            hsl = nat[:, t0:t1, 64 * h : 64 * (h + 1)]
            sq = stat.tile([128, nt, 64], f32, tag=f"sq{len(jbs)}")
            nc.scalar.square(sq[:], hsl)
            ssum = stat.tile([128, nt], f32, tag=f"ssum{len(jbs)}")
            ssq = stat.tile([128, nt], f32, tag=f"ssq{len(jbs)}")
            nc.vector.tensor_reduce(
                ssum[:], hsl, axis=mybir.AxisListType.X, op=addop)
            nc.vector.tensor_reduce(
                ssq[:], sq[:], axis=mybir.AxisListType.X, op=addop)
            mean = stat.tile([128, nt], f32, tag=f"mean{len(jbs)}")
            rstd = stat.tile([128, nt], f32, tag=f"rstd{len(jbs)}")
            var = stat.tile([128, nt], f32, tag=f"var{len(jbs)}")
            nc.vector.tensor_scalar_mul(mean[:], ssum[:], 1.0 / H)
            nc.vector.tensor_mul(var[:], mean[:], mean[:])
            nc.vector.scalar_tensor_tensor(
                out=var[:], in0=ssq[:], scalar=1.0 / H, in1=var[:],
                op0=mult, op1=sub)
            nc.scalar.activation(var[:], var[:], Sqrt, bias=eps_sb[:])
            nc.vector.reciprocal(rstd[:], var[:])
            for jb in jbs:
                tp = tp_ps.tile([64, 512], f16)
                for tt in range(4):
                    t = 4 * jb + tt
                    qn = qn_pool.tile([128, 64], f16)
                    nc.vector.tensor_scalar(
                        out=qn[:], in0=nat[:, t, 64 * h : 64 * h + 64],
                        scalar1=mean[:, t - t0 : t - t0 + 1],
                        scalar2=rstd[:, t - t0 : t - t0 + 1],
                        op0=sub, op1=mult)
                    nc.tensor.transpose(
                        tp[:, 128 * tt : 128 * (tt + 1)], qn[:], id_sb[:])
                dsl = dst_h[h][jb][:]
                if apply_gb:
                    nc.vector.tensor_scalar(
                        out=dsl, in0=tp[:],
                        scalar1=gb_sb[:, 2 * side : 2 * side + 1],
                        scalar2=gb_sb[:, 2 * side + 1 : 2 * side + 2],
                        op0=mult, op1=addop)
                else:
                    nc.any.tensor_copy(dsl, tp[:])

        with tc.tile_pool(name="stat", bufs=2) as stat, \
             tc.tile_pool(name="qn", bufs=6) as qn_pool, \
             tc.tile_pool(name="pt", bufs=4) as pt_pool, \
             tc.tile_pool(name="rr", bufs=2) as rr_pool:
            # ===== Phase 1: projections; q-side LN overlaps the KV pass ====
            with tc.tile_pool(name="proj_ps", bufs=2, space="PSUM") as proj_ps, \
                 tc.tile_pool(name="tp_ps", bufs=2, space="PSUM") as tp_ps:
                for t in range(ST):
                    psq = proj_ps.tile([128, 192], f32, tag="psq")
                    sl = slice(128 * t, 128 * (t + 1))
                    for kt in range(KT):
                        nc.tensor.matmul(
                            psq[:], xq_t[kt][:, sl], w_t[kt][:, 0:192],
                            start=(kt == 0), stop=(kt == KT - 1))
                    nc.any.tensor_copy(q_nat[:, t, :], psq[:])
                for h in range(HPC):
                    ln_head(stat, qn_pool, tp_ps, 0, q_nat, qt_g, h)
                for t in range(ST):
                    pskv = proj_ps.tile([128, 384], f32, tag="pskv")
                    sl = slice(128 * t, 128 * (t + 1))
                    for kt in range(KT):
                        nc.tensor.matmul(
                            pskv[:], xkv_t[kt][:, sl], w_t[kt][:, 192:576],
                            start=(kt == 0), stop=(kt == KT - 1))
                    nc.any.tensor_copy(k_nat[:, t, :], pskv[:, 0:192])
                    nc.any.tensor_copy(
                        vv_sb[:, t, :, 0:64],
                        pskv[:, 192:384].rearrange("p (h d) -> p h d", h=HPC))
                # k-side LN for s-tiles 0..7 (all j=0 attention needs) --
                # overlaps the tail of the kv pass; half1 follows for j=1
                for h in range(HPC):
                    ln_head(stat, qn_pool, tp_ps, 1, k_nat, kt_g, h,
                            jbs=(0, 1))
                for h in range(HPC):
                    ln_head(stat, qn_pool, tp_ps, 1, k_nat, kt_g, h,
                            jbs=(2, 3))

            # ===== Phase 2a: attention (k-side LN already emitted) =====
            with tc.tile_pool(name="st_ps", bufs=3, space="PSUM") as st_ps, \
                 tc.tile_pool(name="zp_ps", bufs=1, space="PSUM") as zp_ps:

                def attn_block(j, h):
                    q0 = QB * j
                    zp = zp_ps.tile([65, QB], f32)
                    n_kt = (j + 1) * KPB

                    def emit_scores(i):
                        off = max(0, 128 * i - q0)   # first valid col
                        diag = 128 * i >= q0
                        stp = st_ps.tile([128, QB], f32)
                        for b0 in range(0, QB, 512):
                            c0, c1 = max(off, b0), b0 + 512
                            if c0 >= c1:
                                continue
                            mask_here = diag and b0 <= off < b0 + 512
                            qg, qo = (q0 + c0) // 512, (q0 + c0) % 512
                            if mask_here:
                                # mask first: depends only on constants, so it
                                # runs off the critical path; the scores matmul
                                # then accumulates onto it (start=False adds
                                # where has_written is set, overwrites the rest)
                                nc.tensor.matmul(
                                    stp[:, off : off + 128], tm_sb[:], id_sb[:],
                                    start=True, stop=False,
                                    skip_group_check=True)
                            nc.tensor.matmul(
                                stp[:, c0:c1],
                                kt_g[h][i // 4][:, 128 * (i % 4) : 128 * (i % 4 + 1)],
                                qt_g[h][qg][:, qo : qo + (c1 - c0)],
                                start=not mask_here, stop=True,
                                skip_group_check=True)
                        return stp, off

                    def emit_exp_pv(i, stp, off):
                        pt = pt_pool.tile([128, QB], bf16)
                        nc.scalar.activation(pt[:, off:], stp[:, off:], Exp)
                        for b0 in range(0, QB, 512):
                            c0, c1 = max(off, b0), b0 + 512
                            if c0 >= c1:
                                continue
                            last_i = min(n_kt, j * KPB + b0 // 128 + 4) - 1
                            nc.tensor.matmul(
                                zp[:, c0:c1], vv_sb[:, i, h, :], pt[:, c0:c1],
                                start=(i == 0), stop=(i == last_i),
                                skip_group_check=True)

                    # software pipeline: scores(i) issued before exp/PV(i-1)
                    prev = None
                    for i in range(n_kt):
                        cur = emit_scores(i)
                        if prev is not None:
                            emit_exp_pv(prev[0], prev[1], prev[2])
                        prev = (i, cur[0], cur[1])
                    emit_exp_pv(prev[0], prev[1], prev[2])
                    # denominators: evict row 64 (r computed in the tail)
                    nc.vector.tensor_copy(l_hj[h * NQB + j][:], zp[64:65, :])
                    # unnormalized Z~^T -> SBUF fp32
                    zdst = (zu01_j[j][64 * h : 64 * (h + 1), :]
                            if h < 2 else zu2_j[j][:, :])
                    nc.any.tensor_copy(zdst, zp[0:64, :])

                for h in range(HPC):
                    attn_block(0, h)
                for h in range(HPC):
                    attn_block(1, h)

        # ====== Phase 2b + 3: normalize Z^T and output projection ======
        # one reciprocal chain per j: l rows -> PE outer products -> [128, 24]
        # columns -> DVE reciprocal -> PE transpose -> selector-matmul bcast
        with tc.tile_pool(name="lc_ps", bufs=1, space="PSUM") as lc_ps, \
             tc.tile_pool(name="r8_ps", bufs=1, space="PSUM") as r8_ps, \
             tc.tile_pool(name="rbc_ps", bufs=2, space="PSUM") as rbc_ps, \
             tc.tile_pool(name="op_ps", bufs=2, space="PSUM") as op_ps, \
             tc.tile_pool(name="rr2", bufs=2) as rr2_pool, \
             tc.tile_pool(name="ot", bufs=4) as ot_pool:
            for j in range(NQB):
                lcol = lc_ps.tile([128, 24], f32)
                for h in range(HPC):
                    ll = l_hj[h * NQB + j]
                    for b in range(8):
                        nc.tensor.matmul(
                            lcol[:, 8 * h + b : 8 * h + b + 1],
                            ll[0:1, 128 * b : 128 * (b + 1)], ones1_sb[:],
                            start=True, stop=True, skip_group_check=True)
                rsbf = rr2_pool.tile([128, 24], f32, tag="rsbf")
                rsb = rr2_pool.tile([128, 24], bf16, tag="rsb")
                nc.vector.reciprocal(rsbf[:], lcol[:])
                nc.vector.tensor_copy(rsb[:], rsbf[:])
                r8p = r8_ps.tile([24, 128], bf16)
                nc.tensor.transpose(r8p[:], rsb[:], idb_sb[:])
                r8 = rr2_pool.tile([24, 128], bf16, tag="r8")
                nc.vector.tensor_copy(r8[:], r8p[:])
                for h in range(HPC):
                    p0 = 64 * (h % 2)
                    zsrc = (zu01_j[j][64 * h : 64 * (h + 1), :]
                            if h < 2 else zu2_j[j][:, :])
                    for half in range(2):  # 512-wide chunks, 1 psum bank each
                        cs = slice(512 * half, 512 * (half + 1))
                        zdst = (z01_jh[j][half][64 * h : 64 * (h + 1), :]
                                if h < 2 else z2_jh[j][half][:, :])
                        rbc = rbc_ps.tile([128, 512], f32)
                        for bb in range(4):
                            b = 8 * h + 4 * half + bb
                            nc.tensor.matmul(
                                rbc[p0 : p0 + 64, 128 * bb : 128 * (bb + 1)],
                                sel_sb[:, b, :], r8[:],
                                start=True, stop=True, skip_group_check=True)
                        nc.vector.tensor_tensor(
                            out=zdst, in0=zsrc[:, cs],
                            in1=rbc[p0 : p0 + 64, :], op=mult)
                # output projection for this j's s-tiles
                for mm in range(QB // 128):
                    m = j * (QB // 128) + mm
                    sl = slice(128 * m, 128 * (m + 1))
                    op = op_ps.tile([128, D], f32)
                    zh, zo = mm // 4, 128 * (mm % 4)
                    for c0 in range(0, D, 512):
                        c1 = min(c0 + 512, D)
                        nc.tensor.matmul(
                            op[:, c0:c1], z01_jh[j][zh][:, zo : zo + 128],
                            wo01_sb[:, c0:c1], start=True, stop=False)
                        nc.tensor.matmul(
                            op[:, c0:c1], z2_jh[j][zh][:, zo : zo + 128],
                            wo2_sb[:, c0:c1], start=False, stop=True)
                    ot = ot_pool.tile([128, D], f32)
                    nc.scalar.copy(ot[:], op[:])
                    nc.sync.dma_start(out=outp[sl, :], in_=ot[:])

        if dump:
            for name, t in [
                ("d_qnat", q_nat), ("d_knat", k_nat), ("d_vv", vv_sb),
            ]:
                dt = nc.dram_tensor(name, list(t.shape), t.dtype,
                                    kind="ExternalOutput")
                nc.sync.dma_start(out=dt[:], in_=t[:])
            for pre, tl in [("d_zu01", zu01_j),
                            ("d_zu2", zu2_j)]:
                for i, t in enumerate(tl):
                    dt = nc.dram_tensor(f"{pre}_{i}", list(t.shape), t.dtype,
                                        kind="ExternalOutput")
                    nc.sync.dma_start(out=dt[:], in_=t[:])

    _split_multi_waits(nc, mybir)
    return nc


def _get_nc(apply_gb: bool):
    key = ("nc", apply_gb)
    if key not in _cache:
        _cache[key] = _build(apply_gb)
    return _cache[key]


def _in_maps(x_q, x_kv, W_Q, W_K, W_V, W_O, ln1_g, ln1_b, ln2_g, ln2_b, apply_gb):
    import ml_dtypes

    bf16 = ml_dtypes.bfloat16
    f16 = np.float16
    ident = np.eye(128, dtype=np.float32).astype(f16)
    identb = np.eye(128, dtype=np.float32).astype(bf16)
    tm = np.where(np.triu(np.ones((128, 128), dtype=bool), 1), NEG, 0.0).astype(f16)
    sel24 = np.ascontiguousarray(
        np.repeat(np.eye(24, dtype=np.float32)[:, :, None], 64, axis=2)).astype(bf16)
    maps = []
    for c in range(N_CORES):
        b, g = divmod(c, 4)
        hs = slice(HPC * g, HPC * (g + 1))
        wq = W_Q[hs].transpose(1, 0, 2).reshape(D, HPC * H)
        wk = W_K[hs].transpose(1, 0, 2).reshape(D, HPC * H)
        wv = W_V[hs].transpose(1, 0, 2).reshape(D, HPC * H)
        m = {
            "xqT": np.ascontiguousarray(x_q[b].T).astype(f16).reshape(KT, 128, S),
            "xkvT": np.ascontiguousarray(x_kv[b].T).astype(f16).reshape(KT, 128, S),
            "wqkv": np.ascontiguousarray(np.concatenate([wq, wk, wv], axis=1))
                      .astype(f16).reshape(KT, 128, 576),
            "wo01": np.ascontiguousarray(W_O[hs][0:2].reshape(128, D)).astype(f16),
            "wo2": np.ascontiguousarray(W_O[hs][2]).astype(f16),
            "ident": ident,
            "identb": identb,
            "tmask": tm,
            "seld": sel24,
        }
        if apply_gb:
            m["lngb"] = np.stack([ln1_g, ln1_b, ln2_g, ln2_b]) \
                          .astype(np.float32).reshape(4, 64, 1)
        maps.append(m)
    return maps


def _run(inputs, trace=False):
    from concourse.bass_utils import run_bass_kernel_spmd

    x_q = np.asarray(inputs["x_q"], np.float32)
    x_kv = np.asarray(inputs["x_kv"], np.float32)
    W_Q = np.asarray(inputs["W_Q"], np.float32)
    W_K = np.asarray(inputs["W_K"], np.float32)
    W_V = np.asarray(inputs["W_V"], np.float32)
    W_O = np.asarray(inputs["W_O"], np.float32)
    g1 = np.asarray(inputs["ln1_g"], np.float32)
    b1 = np.asarray(inputs["ln1_b"], np.float32)
    g2 = np.asarray(inputs["ln2_g"], np.float32)
    b2 = np.asarray(inputs["ln2_b"], np.float32)

    apply_gb = not (
        np.all(g1 == 1) and np.all(g2 == 1) and np.all(b1 == 0) and np.all(b2 == 0)
    )
    nc = _get_nc(apply_gb)
    maps = _in_maps(x_q, x_kv, W_Q, W_K, W_V, W_O, g1, b1, g2, b2, apply_gb)
    res = run_bass_kernel_spmd(nc, maps, list(range(N_CORES)), trace=trace)
    out = np.zeros((B, S, D), np.float32)
    for c in range(N_CORES):
        out[c // 4] += np.asarray(res.results[c]["outp"], np.float32)
    return out, res


def kernel(x_q, x_kv, mask, W_Q, W_K, W_V, W_O, ln1_g, ln1_b, ln2_g, ln2_b):
    out, _ = _run(dict(
        x_q=x_q, x_kv=x_kv, W_Q=W_Q, W_K=W_K, W_V=W_V, W_O=W_O,
        ln1_g=ln1_g, ln1_b=ln1_b, ln2_g=ln2_g, ln2_b=ln2_b))
    return out


def kernel_profiled(x_q, x_kv, mask, W_Q, W_K, W_V, W_O,
                    ln1_g, ln1_b, ln2_g, ln2_b):
    """Runs on HW for correctness; returns (out, exec_time_ns).

    exec_time_ns is the per-core device execution time from the official
    Bass instruction-cost timeline simulator (concourse.timeline_sim) --
    this axon-tunneled rig exposes no NTFF/neuron-profile path, and all 8
    cores run the same program in parallel, so kernel time = per-core time.
    """
    out, res = _run(dict(
        x_q=x_q, x_kv=x_kv, W_Q=W_Q, W_K=W_K, W_V=W_V, W_O=W_O,
        ln1_g=ln1_g, ln1_b=ln1_b, ln2_g=ln2_g, ln2_b=ln2_b))
    from concourse.timeline_sim import TimelineSim
    g1 = np.asarray(ln1_g); b1 = np.asarray(ln1_b)
    g2 = np.asarray(ln2_g); b2 = np.asarray(ln2_b)
    apply_gb = not (np.all(g1 == 1) and np.all(g2 == 1)
                    and np.all(b1 == 0) and np.all(b2 == 0))
    exec_ns = TimelineSim(_get_nc(apply_gb), trace=False).simulate()
    return out, int(exec_ns)
